# revision 2
# baseline (speedup 1.0000x reference)
"""Trainium2 Bass kernel for the DNPU local-receptive-field surrogate model
(fp16, host-preshuffled x, output layer piggybacked on layer 0, unpaired
single-bank PSUM drains).

Model (see reference): x [B,1,64,64] -> 2x2/stride-2 unfold -> per-node
7-electrode assembly (4 data + 3 control electrodes, placements given by
data_idx/ctrl_idx) -> shared MLP 7->90->(90x4)->1 -> out [B,32,32].

Strategy. The PE streams one moving column per 1.2GHz cycle regardless of
dtype on this part, so 512-column matmul passes are the scarce resource;
each token goes through the net in FIVE passes:
  - Host pre-shuffles x into a 2x2-parity split layout and packs it with
    the batch-replicated controls^T into a [7, N_NODES] per-batch carrier
    block, DMA'd into partitions 90-96 of the previous pair's last hidden
    tile.
  - Layer 0 and the OUTPUT layer share one matmul: stationary [97, 97]
    has x/ctrl rows (k=90..96) feeding columns 0-89 (layer-0 units) and
    W_out (k=0..89, reading the carrier tile's h4 rows) feeding column 96
    (32-aligned so the drain AP start is legal). One pass computes u0 for
    pair q and 512 output values of pair q-5.
  - Tokens on the free dim, hidden units on partitions. Pairs of 512-node
    tiles (one batch image) run back-to-back matmuls into two separate
    single-bank PSUM tiles, each drained by its own [.,512] bias+ReLU op
    with the two drains split across ACT and DVE (the only PSUM-capable
    engines). Single-bank tiles keep 8 PSUM buffers in rotation, giving
    the PE sequencer enough runway that LDWEIGHTS stays overlapped.
  - Software pipelining: pair q runs stage s at wave q+s (5 stages).
  - Tail: the last SKEW pairs' outputs ride on dummy stage-0 passes
    (garbage layer-0 rows are never drained), avoiding a serial tail.
"""

import numpy as np

import concourse.bass as bass
import concourse.mybir as mybir
import concourse.tile as _tile
from concourse.bass_utils import run_bass_kernel_spmd

# ---------------------------------------------------------------------------
# Workaround: this neuronxcc walrus build rejects instructions carrying more
# than one sem wait ("Too many sync wait commands"). After scheduling, spill
# excess waits onto NOPs inserted just before the instruction on the same
# engine.
_MAX_SYNC_WAITS = 1
_nop_counter = [0]


def _split_excess_sync_waits(nc, maxw=_MAX_SYNC_WAITS):
    for f in nc.m.functions:
        for bb in f.blocks:
            insts = list(bb.instructions)
            if not any(
                ins.sync_info is not None and len(ins.sync_info.on_wait or []) > maxw
                for ins in insts
            ):
                continue
            new = []
            for ins in insts:
                si = ins.sync_info
                waits = list(si.on_wait or []) if si is not None else []
                if len(waits) > maxw:
                    excess, keep = waits[: len(waits) - maxw], waits[-maxw:]
                    for i in range(0, len(excess), maxw):
                        _nop_counter[0] += 1
                        nop = mybir.InstNoOp(name=f"waitsplit_{_nop_counter[0]}")
                        nop.engine = ins.engine
                        nop.sync_info = mybir.SyncInfo(
                            on_wait=excess[i : i + maxw], on_update=[]
                        )
                        new.append(nop)
                    si.on_wait = keep
                new.append(ins)
            bb.instructions = new

# ---------------------------------------------------------------------------
# Problem constants (hardcoded per the task contract).
B = 512
H = W = 64
N_NODES = 1024
HID = 90
N_HIDDEN = 4
N_CORES = 8
B_CORE = B // N_CORES  # 64 batches per core

N_TILE = 512  # tokens per matmul (hard ISA cap on the moving free dim)
N_PAIRS = B_CORE  # one pair (2 tiles) per batch image
N_STAGE = 5  # l0(+carried out), h1..h4
CARRY = 7  # x/ctrl carrier rows appended to the hidden tiles
HP = HID + CARRY  # 97 partitions in hidden tiles
SKEW = 5  # the out row of pair q materializes in pair q+SKEW's stage 0
OUT_P = 96  # out column lands on partition 96 (engine APs need 32-aligned starts)

F32 = mybir.dt.float32
F16 = mybir.dt.float16

Relu = mybir.ActivationFunctionType.Relu
Identity = mybir.ActivationFunctionType.Identity
ALU_ADD = mybir.AluOpType.add
ALU_MAX = mybir.AluOpType.max


def _build_program():
    """Trace the per-core Bass program (identical on all 8 cores)."""
    nc = bass.Bass()

    # per-batch carrier blocks: 4 x-parity rows + 3 control rows
    xc_d = nc.dram_tensor("xc", [B_CORE, CARRY, N_NODES], F16, kind="ExternalInput")
    z_d = nc.dram_tensor("z", [HID, N_NODES], F16, kind="ExternalInput")
    w5_d = nc.dram_tensor("w5", [HP, OUT_P + 1], F16, kind="ExternalInput")
    wh_d = nc.dram_tensor("wh", [N_HIDDEN, HID, HID], F16, kind="ExternalInput")
    bin_d = nc.dram_tensor("bin", [HID], F32, kind="ExternalInput")
    bh_d = nc.dram_tensor("bh", [N_HIDDEN, HID], F32, kind="ExternalInput")
    bo_d = nc.dram_tensor("bo", [1], F32, kind="ExternalInput")
    out_d = nc.dram_tensor("out", [B_CORE, N_NODES], F32, kind="ExternalOutput")

    # greedy drain load balancer (expected per-op engine-busy ns, measured)
    busy = {"a": 0.0, "v": 0.0}
    cost = {"a": 700.0, "v": 790.0}

    def pick():
        e = min(("a", "v"), key=lambda k: busy[k] + cost[k])
        busy[e] += cost[e]
        return e

    with _tile.TileContext(nc) as tc:
        with (
            tc.tile_pool(name="const", bufs=1) as const,
            tc.tile_pool(name="oop", bufs=3) as oop,
            tc.tile_pool(name="hbuf", bufs=10) as hbuf,
            tc.tile_pool(name="ps", bufs=8, space="PSUM") as ps,
        ):
            # ---- constants ----
            w5 = const.tile([HP, OUT_P + 1], F16)
            nc.sync.dma_start(w5[:], w5_d[:])
            wh = const.tile([HID, N_HIDDEN, HID], F16)
            nc.sync.dma_start(wh[:], wh_d[:].rearrange("i a b -> a i b"))
            bin_t = const.tile([HID, 1], F32)
            nc.sync.dma_start(bin_t[:], bin_d[:].unsqueeze(1))
            bh_t = const.tile([HID, N_HIDDEN], F32)
            nc.sync.dma_start(bh_t[:], bh_d[:].rearrange("i h -> h i"))
            bo_t = const.tile([1, 1], F32)
            nc.sync.dma_start(bo_t[:], bo_d[:].unsqueeze(0))

            def drain(dst, src, bias):
                if pick() == "a":
                    nc.scalar.activation(dst, src, Relu, bias=bias)
                else:
                    nc.vector.tensor_scalar(
                        out=dst, in0=src, scalar1=bias, scalar2=0.0,
                        op0=ALU_ADD, op1=ALU_MAX,
                    )

            def out_drain(dst, src):
                if pick() == "a":
                    nc.scalar.activation(dst, src, Identity, bias=bo_t[0:1, 0:1])
                else:
                    nc.vector.tensor_scalar(
                        out=dst, in0=src, scalar1=bo_t[0:1, 0:1],
                        scalar2=None, op0=ALU_ADD,
                    )

            h_cur = {}  # pair q -> live hidden SBUF tile [97, 1024]
            carrier = {}  # pair q -> tile whose rows 90:97 hold pair q's x/ctrl

            # ---- warm-up carriers for pairs 0..SKEW-1 (rows 0-89 zero-filled
            # by DMA so the unused out column sees no NaNs; the l0 columns
            # read only rows 90-96) ----
            for q in range(SKEW):
                dm = hbuf.tile([HP, N_NODES], F16, tag="h", name="dm")
                nc.sync.dma_start(dm[0:HID, :], z_d[:])
                nc.sync.dma_start(dm[HID:HP, :], xc_d[q])
                carrier[q] = dm

            def emit_stage0(q, dummy=False):
                """Combined l0 + carried-out pass. q may be a virtual pair
                index >= N_PAIRS (dummy=True): only the out row is used."""
                cr = carrier.pop(q) if not dummy else h_cur[q - SKEW]
                pins = []
                for u in range(2):
                    o0 = u * N_TILE
                    pin = ps.tile([OUT_P + 1, N_TILE], F32, tag="ps", name="pin")
                    nc.tensor.matmul(
                        pin[:], w5[:], cr[:, o0 : o0 + N_TILE],
                        start=True, stop=True,
                    )
                    pins.append(pin)
                if not dummy:
                    h = hbuf.tile([HP, N_NODES], F16, tag="h", name="h")
                    for u in range(2):
                        o0 = u * N_TILE
                        drain(h[0:HID, o0 : o0 + N_TILE], pins[u][0:HID, :],
                              bin_t[:, 0:1])
                    h_cur[q] = h
                if q >= SKEW:  # pair q-SKEW's output came along for free
                    qo = q - SKEW
                    oo = oop.tile([1, N_NODES], F32, tag="oo", name="oo")
                    for u in range(2):
                        o0 = u * N_TILE
                        out_drain(oo[0:1, o0 : o0 + N_TILE],
                                  pins[u][OUT_P : OUT_P + 1, :])
                    nc.sync.dma_start(out_d[qo : qo + 1, :], oo[:])
                    if dummy:
                        h_cur.pop(q - SKEW)

            def emit_stage(q, s):
                if s == 0:
                    emit_stage0(q)
                    return
                li = s - 1
                hp = h_cur[q]
                h = hbuf.tile([HP, N_NODES], F16, tag="h", name="h")
                for u in range(2):
                    o0 = u * N_TILE
                    pin = ps.tile([OUT_P + 1, N_TILE], F32, tag="ps", name="pin")
                    nc.tensor.matmul(
                        pin[0:HID, :], wh[:, li, :], hp[0:HID, o0 : o0 + N_TILE]
                    )
                    drain(h[0:HID, o0 : o0 + N_TILE], pin[0:HID, :],
                          bh_t[:, li : li + 1])
                if s == N_HIDDEN:
                    # this tile carries pair q+SKEW's x/ctrl block
                    if q + SKEW < N_PAIRS:
                        nc.sync.dma_start(h[HID:HP, :], xc_d[q + SKEW])
                        carrier[q + SKEW] = h
                h_cur[q] = h

            for w in range(N_PAIRS + N_STAGE - 1 + SKEW):
                for s in range(N_STAGE - 1, -1, -1):  # oldest pair first
                    q = w - s
                    if 0 <= q < N_PAIRS:
                        emit_stage(q, s)
                # dummy stage-0 passes carry the last pairs' outputs
                q = w - (N_STAGE - 1)
                if N_PAIRS <= q < N_PAIRS + SKEW:
                    emit_stage0(q, dummy=True)

    _split_excess_sync_waits(nc)
    return nc


def _prep_weights(controls, W_in, b_in, W_h, b_h, W_out, b_out, data_idx, ctrl_idx):
    """Host-side gather of W_in rows per electrode placement (replicating the
    reference's scatter semantics: controls overwrite colliding data slots)."""
    di = np.asarray(data_idx)[0].tolist()  # placements identical across nodes
    ci = np.asarray(ctrl_idx)[0].tolist()
    W_in = np.asarray(W_in, dtype=np.float32)
    Wd = W_in[di, :].copy()  # [4, HID]
    cset = set(ci)
    for j in range(4):
        if di[j] in cset or di[j] in di[j + 1 :]:
            Wd[j] = 0.0  # overwritten by a control (or a later data) electrode
    Wc = W_in[ci, :].copy()  # [3, HID]
    for k in range(3):
        if ci[k] in ci[k + 1 :]:
            Wc[k] = 0.0  # later control write wins

    w0 = np.concatenate([Wd, Wc], axis=0)  # [7, HID], (kh,kw)-order + controls
    wo = np.asarray(W_out, np.float32)  # [HID, 1]

    # combined layer0+output stationary [97, 97]:
    #   k rows 90..96 (carrier x/ctrl) -> columns 0..89 (layer-0 units)
    #   k rows 0..89 (carrier h4)      -> column 96     (output row; 32-aligned
    #   partition so the drain AP start is legal; columns 90-95 are zero)
    w5 = np.zeros((HP, OUT_P + 1), np.float32)
    w5[HID:HP, 0:HID] = w0
    w5[0:HID, OUT_P] = wo[:, 0]

    return {
        "w5": w5.astype(np.float16),
        "wh": np.ascontiguousarray(np.asarray(W_h, np.float16)),
        "bin": np.ascontiguousarray(np.asarray(b_in, np.float32)),
        "bh": np.ascontiguousarray(np.asarray(b_h, np.float32)),
        "bo": np.ascontiguousarray(np.asarray(b_out, np.float32)),
        "z": np.zeros((HID, N_NODES), np.float16),
        "ctrlT": np.asarray(controls, np.float32).T,  # [3, N_NODES]
    }


def _run(inputs, trace=False):
    x = np.asarray(inputs["x"], dtype=np.float32)
    common = _prep_weights(
        inputs["controls"],
        inputs["W_in"],
        inputs["b_in"],
        inputs["W_h"],
        inputs["b_h"],
        inputs["W_out"],
        inputs["b_out"],
        inputs["data_idx"],
        inputs["ctrl_idx"],
    )
    ctrlT = common.pop("ctrlT")

    # host pre-shuffle: xp[b, 2*kh+kw, r*32+c2] = x[b, 0, 2r+kh, 2c2+kw],
    # packed with the controls into per-batch [7, N_NODES] carrier blocks
    xv = x[:, 0].reshape(B, H // 2, 2, W // 2, 2)  # [B, r, kh, c2, kw]
    xp = xv.transpose(0, 2, 4, 1, 3).reshape(B, 4, N_NODES)
    xc = np.empty((B, CARRY, N_NODES), np.float16)
    xc[:, 0:4] = xp.astype(np.float16)
    xc[:, 4:7] = ctrlT.astype(np.float16)[None]

    nc = _build_program()

    core_ids = list(range(N_CORES))
    in_maps = []
    for i in core_ids:
        shard = np.ascontiguousarray(xc[i * B_CORE : (i + 1) * B_CORE])
        in_maps.append({"xc": shard, **common})

    res = run_bass_kernel_spmd(nc, in_maps, core_ids, trace=trace)
    globals()["_last_res"] = res
    out = np.concatenate([res.results[i]["out"] for i in core_ids], axis=0)
    return out.reshape(B, 32, 32), res.exec_time_ns


def kernel(**inputs):
    return _run(inputs, trace=False)[0]


# revision 3
# speedup vs baseline: 1.0062x; 1.0062x over previous
"""Trainium2 Bass kernel for the DNPU local-receptive-field surrogate model
(fp16, host-preshuffled x, output layer piggybacked on layer 0, unpaired
single-bank PSUM drains).

Model (see reference): x [B,1,64,64] -> 2x2/stride-2 unfold -> per-node
7-electrode assembly (4 data + 3 control electrodes, placements given by
data_idx/ctrl_idx) -> shared MLP 7->90->(90x4)->1 -> out [B,32,32].

Strategy. The PE streams one moving column per 1.2GHz cycle regardless of
dtype on this part, so 512-column matmul passes are the scarce resource;
each token goes through the net in FIVE passes:
  - Host pre-shuffles x into a 2x2-parity split layout and packs it with
    the batch-replicated controls^T into a [7, N_NODES] per-batch carrier
    block, DMA'd into partitions 90-96 of the previous pair's last hidden
    tile.
  - Layer 0 and the OUTPUT layer share one matmul: stationary [97, 97]
    has x/ctrl rows (k=90..96) feeding columns 0-89 (layer-0 units) and
    W_out (k=0..89, reading the carrier tile's h4 rows) feeding column 96
    (32-aligned so the drain AP start is legal). One pass computes u0 for
    pair q and 512 output values of pair q-5.
  - Tokens on the free dim, hidden units on partitions. Pairs of 512-node
    tiles (one batch image) run back-to-back matmuls into two separate
    single-bank PSUM tiles, each drained by its own [.,512] bias+ReLU op
    with the two drains split across ACT and DVE (the only PSUM-capable
    engines). Single-bank tiles keep 8 PSUM buffers in rotation, giving
    the PE sequencer enough runway that LDWEIGHTS stays overlapped.
  - Software pipelining: pair q runs stage s at wave q+s (5 stages).
  - Tail: the last SKEW pairs' outputs ride on dummy stage-0 passes
    (garbage layer-0 rows are never drained), avoiding a serial tail.
"""

import numpy as np

import concourse.bass as bass
import concourse.mybir as mybir
import concourse.tile as _tile
from concourse.bass_utils import run_bass_kernel_spmd

# ---------------------------------------------------------------------------
# Workaround: this neuronxcc walrus build rejects instructions carrying more
# than one sem wait ("Too many sync wait commands"). After scheduling, spill
# excess waits onto NOPs inserted just before the instruction on the same
# engine.
_MAX_SYNC_WAITS = 1
_nop_counter = [0]


def _split_excess_sync_waits(nc, maxw=_MAX_SYNC_WAITS):
    for f in nc.m.functions:
        for bb in f.blocks:
            insts = list(bb.instructions)
            if not any(
                ins.sync_info is not None and len(ins.sync_info.on_wait or []) > maxw
                for ins in insts
            ):
                continue
            new = []
            for ins in insts:
                si = ins.sync_info
                waits = list(si.on_wait or []) if si is not None else []
                if len(waits) > maxw:
                    excess, keep = waits[: len(waits) - maxw], waits[-maxw:]
                    for i in range(0, len(excess), maxw):
                        _nop_counter[0] += 1
                        nop = mybir.InstNoOp(name=f"waitsplit_{_nop_counter[0]}")
                        nop.engine = ins.engine
                        nop.sync_info = mybir.SyncInfo(
                            on_wait=excess[i : i + maxw], on_update=[]
                        )
                        new.append(nop)
                    si.on_wait = keep
                new.append(ins)
            bb.instructions = new

# ---------------------------------------------------------------------------
# Problem constants (hardcoded per the task contract).
B = 512
H = W = 64
N_NODES = 1024
HID = 90
N_HIDDEN = 4
N_CORES = 8
B_CORE = B // N_CORES  # 64 batches per core

N_TILE = 512  # tokens per matmul (hard ISA cap on the moving free dim)
N_PAIRS = B_CORE  # one pair (2 tiles) per batch image
N_STAGE = 5  # l0(+carried out), h1..h4
CARRY = 7  # x/ctrl carrier rows appended to the hidden tiles
HP = HID + CARRY  # 97 partitions in hidden tiles
SKEW = 5  # the out row of pair q materializes in pair q+SKEW's stage 0
OUT_P = 96  # out column lands on partition 96 (engine APs need 32-aligned starts)

F32 = mybir.dt.float32
F16 = mybir.dt.float16

Relu = mybir.ActivationFunctionType.Relu
Identity = mybir.ActivationFunctionType.Identity
ALU_ADD = mybir.AluOpType.add
ALU_MAX = mybir.AluOpType.max


def _build_program():
    """Trace the per-core Bass program (identical on all 8 cores)."""
    nc = bass.Bass()

    # per-batch carrier blocks: 4 x-parity rows + 3 control rows
    xc_d = nc.dram_tensor("xc", [B_CORE, CARRY, N_NODES], F16, kind="ExternalInput")
    w5_d = nc.dram_tensor("w5", [HP, OUT_P + 1], F16, kind="ExternalInput")
    w0s_d = nc.dram_tensor("w0s", [CARRY, HID], F16, kind="ExternalInput")
    wh_d = nc.dram_tensor("wh", [N_HIDDEN, HID, HID], F16, kind="ExternalInput")
    bin_d = nc.dram_tensor("bin", [HID], F32, kind="ExternalInput")
    bh_d = nc.dram_tensor("bh", [N_HIDDEN, HID], F32, kind="ExternalInput")
    bo_d = nc.dram_tensor("bo", [1], F32, kind="ExternalInput")
    out_d = nc.dram_tensor("out", [B_CORE, N_NODES], F32, kind="ExternalOutput")

    # greedy drain load balancer (expected per-op engine-busy ns, measured)
    busy = {"a": 0.0, "v": 0.0}
    cost = {"a": 700.0, "v": 790.0}

    def pick():
        e = min(("a", "v"), key=lambda k: busy[k] + cost[k])
        busy[e] += cost[e]
        return e

    with _tile.TileContext(nc) as tc:
        with (
            tc.tile_pool(name="const", bufs=1) as const,
            tc.tile_pool(name="oop", bufs=3) as oop,
            tc.tile_pool(name="hbuf", bufs=12) as hbuf,
            tc.tile_pool(name="ps", bufs=8, space="PSUM") as ps,
        ):
            # ---- constants (first-wave dependencies first) ----
            w5 = const.tile([HP, OUT_P + 1], F16)
            nc.sync.dma_start(w5[:], w5_d[:])
            w0s = const.tile([CARRY, HID], F16)
            nc.sync.dma_start(w0s[:], w0s_d[:])
            bin_t = const.tile([HID, 1], F32)
            nc.sync.dma_start(bin_t[:], bin_d[:].unsqueeze(1))
            wh = const.tile([HID, N_HIDDEN, HID], F16)
            nc.sync.dma_start(wh[:], wh_d[:].rearrange("i a b -> a i b"))
            bh_t = const.tile([HID, N_HIDDEN], F32)
            nc.sync.dma_start(bh_t[:], bh_d[:].rearrange("i h -> h i"))
            bo_t = const.tile([1, 1], F32)
            nc.sync.dma_start(bo_t[:], bo_d[:].unsqueeze(0))

            def drain(dst, src, bias):
                if pick() == "a":
                    nc.scalar.activation(dst, src, Relu, bias=bias)
                else:
                    nc.vector.tensor_scalar(
                        out=dst, in0=src, scalar1=bias, scalar2=0.0,
                        op0=ALU_ADD, op1=ALU_MAX,
                    )

            def out_drain(dst, src):
                if pick() == "a":
                    nc.scalar.activation(dst, src, Identity, bias=bo_t[0:1, 0:1])
                else:
                    nc.vector.tensor_scalar(
                        out=dst, in0=src, scalar1=bo_t[0:1, 0:1],
                        scalar2=None, op0=ALU_ADD,
                    )

            h_cur = {}  # pair q -> live hidden SBUF tile [97, 1024]
            carrier = {}  # pair q -> tile whose rows 90:97 hold pair q's x/ctrl

            # ---- warm-up carriers for pairs 0..SKEW-1: small x/ctrl-only
            # tiles; their stage 0 uses the plain [7,90] stationary so no
            # garbage hidden rows are ever read ----
            for q in range(SKEW):
                dm = hbuf.tile([CARRY, N_NODES], F16, tag="wup", bufs=SKEW, name="dm")
                nc.sync.dma_start(dm[:], xc_d[q])
                carrier[q] = dm

            def emit_stage0(q, dummy=False):
                """Combined l0 + carried-out pass. q may be a virtual pair
                index >= N_PAIRS (dummy=True): only the out row is used."""
                cr = carrier.pop(q) if not dummy else h_cur[q - SKEW]
                warm = not dummy and q < SKEW
                pins = []
                for u in range(2):
                    o0 = u * N_TILE
                    pin = ps.tile([OUT_P + 1, N_TILE], F32, tag="ps", name="pin")
                    if warm:
                        nc.tensor.matmul(
                            pin[0:HID, :], w0s[:], cr[:, o0 : o0 + N_TILE],
                            start=True, stop=True,
                        )
                    else:
                        nc.tensor.matmul(
                            pin[:], w5[:], cr[:, o0 : o0 + N_TILE],
                            start=True, stop=True,
                        )
                    pins.append(pin)
                if not dummy:
                    h = hbuf.tile([HP, N_NODES], F16, tag="h", name="h")
                    for u in range(2):
                        o0 = u * N_TILE
                        drain(h[0:HID, o0 : o0 + N_TILE], pins[u][0:HID, :],
                              bin_t[:, 0:1])
                    h_cur[q] = h
                if q >= SKEW:  # pair q-SKEW's output came along for free
                    qo = q - SKEW
                    oo = oop.tile([1, N_NODES], F32, tag="oo", name="oo")
                    for u in range(2):
                        o0 = u * N_TILE
                        out_drain(oo[0:1, o0 : o0 + N_TILE],
                                  pins[u][OUT_P : OUT_P + 1, :])
                    nc.sync.dma_start(out_d[qo : qo + 1, :], oo[:])
                    if dummy:
                        h_cur.pop(q - SKEW)

            def emit_stage(q, s):
                if s == 0:
                    emit_stage0(q)
                    return
                li = s - 1
                hp = h_cur[q]
                h = hbuf.tile([HP, N_NODES], F16, tag="h", name="h")
                for u in range(2):
                    o0 = u * N_TILE
                    pin = ps.tile([OUT_P + 1, N_TILE], F32, tag="ps", name="pin")
                    nc.tensor.matmul(
                        pin[0:HID, :], wh[:, li, :], hp[0:HID, o0 : o0 + N_TILE]
                    )
                    drain(h[0:HID, o0 : o0 + N_TILE], pin[0:HID, :],
                          bh_t[:, li : li + 1])
                if s == N_HIDDEN:
                    # this tile carries pair q+SKEW's x/ctrl block
                    if q + SKEW < N_PAIRS:
                        nc.sync.dma_start(h[HID:HP, :], xc_d[q + SKEW])
                        carrier[q + SKEW] = h
                h_cur[q] = h

            # warmup stage-0 passes have no h4 dependency: burst them all
            # up front so the PE has work while the pipeline fills
            for q in range(SKEW):
                emit_stage(q, 0)

            for w in range(N_PAIRS + SKEW):
                for s in range(N_STAGE - 1, -1, -1):  # oldest pair first
                    q = w - s
                    if 0 <= q < N_PAIRS and not (s == 0 and q < SKEW):
                        emit_stage(q, s)
                # dummy passes carrying the last pairs' outputs run as soon
                # as their h4 carrier exists (wave q), not after the drain
                if N_PAIRS <= w < N_PAIRS + SKEW:
                    emit_stage0(w, dummy=True)

    _split_excess_sync_waits(nc)
    return nc


def _prep_weights(controls, W_in, b_in, W_h, b_h, W_out, b_out, data_idx, ctrl_idx):
    """Host-side gather of W_in rows per electrode placement (replicating the
    reference's scatter semantics: controls overwrite colliding data slots)."""
    di = np.asarray(data_idx)[0].tolist()  # placements identical across nodes
    ci = np.asarray(ctrl_idx)[0].tolist()
    W_in = np.asarray(W_in, dtype=np.float32)
    Wd = W_in[di, :].copy()  # [4, HID]
    cset = set(ci)
    for j in range(4):
        if di[j] in cset or di[j] in di[j + 1 :]:
            Wd[j] = 0.0  # overwritten by a control (or a later data) electrode
    Wc = W_in[ci, :].copy()  # [3, HID]
    for k in range(3):
        if ci[k] in ci[k + 1 :]:
            Wc[k] = 0.0  # later control write wins

    w0 = np.concatenate([Wd, Wc], axis=0)  # [7, HID], (kh,kw)-order + controls
    wo = np.asarray(W_out, np.float32)  # [HID, 1]

    # combined layer0+output stationary [97, 97]:
    #   k rows 90..96 (carrier x/ctrl) -> columns 0..89 (layer-0 units)
    #   k rows 0..89 (carrier h4)      -> column 96     (output row; 32-aligned
    #   partition so the drain AP start is legal; columns 90-95 are zero)
    w5 = np.zeros((HP, OUT_P + 1), np.float32)
    w5[HID:HP, 0:HID] = w0
    w5[0:HID, OUT_P] = wo[:, 0]

    return {
        "w5": w5.astype(np.float16),
        "w0s": np.ascontiguousarray(w0.astype(np.float16)),
        "wh": np.ascontiguousarray(np.asarray(W_h, np.float16)),
        "bin": np.ascontiguousarray(np.asarray(b_in, np.float32)),
        "bh": np.ascontiguousarray(np.asarray(b_h, np.float32)),
        "bo": np.ascontiguousarray(np.asarray(b_out, np.float32)),
        "ctrlT": np.asarray(controls, np.float32).T,  # [3, N_NODES]
    }


def _run(inputs, trace=False):
    x = np.asarray(inputs["x"], dtype=np.float32)
    common = _prep_weights(
        inputs["controls"],
        inputs["W_in"],
        inputs["b_in"],
        inputs["W_h"],
        inputs["b_h"],
        inputs["W_out"],
        inputs["b_out"],
        inputs["data_idx"],
        inputs["ctrl_idx"],
    )
    ctrlT = common.pop("ctrlT")

    # host pre-shuffle: xp[b, 2*kh+kw, r*32+c2] = x[b, 0, 2r+kh, 2c2+kw],
    # packed with the controls into per-batch [7, N_NODES] carrier blocks
    xv = x[:, 0].reshape(B, H // 2, 2, W // 2, 2)  # [B, r, kh, c2, kw]
    xp = xv.transpose(0, 2, 4, 1, 3).reshape(B, 4, N_NODES)
    xc = np.empty((B, CARRY, N_NODES), np.float16)
    xc[:, 0:4] = xp.astype(np.float16)
    xc[:, 4:7] = ctrlT.astype(np.float16)[None]

    nc = _build_program()

    core_ids = list(range(N_CORES))
    in_maps = []
    for i in core_ids:
        shard = np.ascontiguousarray(xc[i * B_CORE : (i + 1) * B_CORE])
        in_maps.append({"xc": shard, **common})

    res = run_bass_kernel_spmd(nc, in_maps, core_ids, trace=trace)
    globals()["_last_res"] = res
    out = np.concatenate([res.results[i]["out"] for i in core_ids], axis=0)
    return out.reshape(B, 32, 32), res.exec_time_ns


def kernel(**inputs):
    return _run(inputs, trace=False)[0]
